# revision 20
# baseline (speedup 1.0000x reference)
"""GPT self-attention (B=4, S=2048, D=1024, H=16) on 8 NeuronCores.

Sharding: core c = (batch b = c//2, head-group g = c%2 of 8 heads).
Each core computes q/k/v projections for its 8 heads, causal attention,
and a partial output projection (rows of w_dense for its heads).
Host sums the two partials per batch (tensor-parallel unshard) + bias.

Schedule: heads are processed in pairs (2j, 2j+1) sharing one key-tile
loop, so the two heads' score matmuls (contraction 64, base partitions
0/64) run concurrently in disjoint PE row groups.  Projection GEMMs are
chopped into ~1.7us work units and drip-fed into the attention phases'
ACT-bound gaps.  Inputs stream per contraction-chunk so projection
matmuls start while x is still loading.  Softmax normalization is
progressive: each 512-column ctx chunk stops accumulating at its own
last key tile, is drained + normalized immediately (rowsum from the
ones-column of V, approx-reciprocal, GpSimd broadcast, one multiply),
and the final phase's output-projection units drip in as their ctx
columns complete, so the tail after the last score matmul is short.
"""

import numpy as np
import ml_dtypes

import concourse.bass as bass
import concourse.mybir as mybir
import concourse.tile as tile
from concourse import bacc
from concourse import bass_utils

B, S, D, H = 4, 2048, 1024, 16
HD = D // H          # 64
NCORES = 8
GH = 8               # heads per core (group)
DG = GH * HD         # 512 dims per group
P = 128
NKT = S // P         # 16 key tiles
NJ = DG // P         # 4 partition-tiles of group dims
NKD = D // P         # 8 contraction tiles for projections
CH = 512             # psum chunk (one bank of f32)
HW = 1024            # q-half width
NPAIR = GH // 2      # 4 head pairs
NC_H = HW // CH      # 2 512-chunks per half

BF16 = mybir.dt.bfloat16
F32 = mybir.dt.float32
NPBF16 = ml_dtypes.bfloat16

_COMPILED = None


def _build_body(tc, aps, dbg=None):
    nc = tc.nc
    xT = aps["xT"].rearrange("(k p) s -> p k s", p=P)      # [128, 8, 2048]
    wq = aps["wq"].rearrange("(k p) m -> p k m", p=P)      # [128, 8, 512]
    wk = aps["wk"].rearrange("(k p) m -> p k m", p=P)
    wv = aps["wv"].rearrange("(k p) m -> p k m", p=P)
    wd = aps["wd"].rearrange("(j p) n -> p j n", p=P)      # [128, 4, 1024]
    maskin = aps["mask"]                                   # [128, 128] bf16
    outp = aps["outp"]                                     # [2048, 1024] f32

    Exp = mybir.ActivationFunctionType.Exp

    with (
        tc.tile_pool(name="const", bufs=1) as cpool,
        tc.tile_pool(name="pts", bufs=4) as ppool,
        tc.tile_pool(name="c64", bufs=4) as cstg,
        tc.tile_pool(name="bc", bufs=3) as bcp,
        tc.tile_pool(name="r0", bufs=4) as r0p,
        tc.tile_pool(name="ost", bufs=2) as ostg,
        tc.tile_pool(name="pssc", bufs=2, space=bass.MemorySpace.PSUM) as psc,
        tc.tile_pool(name="psctx", bufs=2, space=bass.MemorySpace.PSUM) as pcx,
    ):
        # ---- persistent SBUF tensors ----
        xT_t = cpool.tile([P, NKD, S], BF16, tag="xT")
        wq_t = cpool.tile([P, NKD, DG], BF16, tag="wq")
        wk_t = cpool.tile([P, NKD, DG], BF16, tag="wk")
        wv_t = cpool.tile([P, NKD, DG], BF16, tag="wv")
        wd_t = cpool.tile([P, NJ, D], BF16, tag="wd")
        mask_t = cpool.tile([P, P], BF16, tag="mask")
        qT_t = cpool.tile([P, NJ, S], BF16, tag="qT")      # [dim, s]
        kT_t = cpool.tile([P, NJ, S], BF16, tag="kT")
        # v_aug: per s-tile, per head: 64 v-dims + ones column (65 wide)
        v_t = cpool.tile([P, NKT, GH * (HD + 1)], BF16, tag="v")
        ctxT_t = cpool.tile([P, NJ, S], BF16, tag="ctxT")  # normalized ctx^T

        # ---- input DMA: per contraction-chunk, in need order.  All
        # in-flight transfers share the 16 hardware queues round-robin, so
        # issue order ~= delivery order: mask (warmup dep) first, then
        # wq+x interleaved (first projection), then wv/wk, wd last ----
        nc.gpsimd.dma_start(mask_t[:], maskin)
        for kt in range(NKD):
            nc.sync.dma_start(wq_t[:, kt, :], wq[:, kt, :])
            nc.scalar.dma_start(xT_t[:, kt, :], xT[:, kt, :])
        nc.sync.dma_start(wv_t[:], wv)
        for kt in range(NKD):
            nc.scalar.dma_start(wk_t[:, kt, :], wk[:, kt, :])
        nc.scalar.dma_start(wd_t[:], wd)
        # ones columns of v_aug
        v_heads = v_t.rearrange("p t (h c) -> p t h c", c=HD + 1)
        nc.vector.memset(v_heads[:, :, :, HD:], 1.0)

        # ---- projection work units (~1.7us of PE each); kt-outer so the
        # accumulation consumes x chunks in DMA arrival order ----
        def qk_unit(dst, w, j, n0):
            def emit():
                ps = psc.tile([P, 2 * CH], F32, tag="sc")
                for kt in range(NKD):
                    for sub in range(2):
                        nc.tensor.matmul(
                            ps[:, sub * CH:(sub + 1) * CH],
                            w[:, kt, j * P:(j + 1) * P],
                            xT_t[:, kt, n0 + sub * CH:n0 + (sub + 1) * CH],
                            start=(kt == 0), stop=(kt == NKD - 1),
                        )
                nc.vector.tensor_copy(dst[:, j, n0:n0 + 2 * CH], ps[:])
            return emit

        def v_unit(st):
            def emit():
                ps = psc.tile([P, 2 * CH], F32, tag="sc")
                for kt in range(NKD):
                    for sub in range(2):
                        nc.tensor.matmul(
                            ps[:, sub * CH:(sub + 1) * CH],
                            xT_t[:, kt, (st + sub) * P:(st + sub + 1) * P],
                            wv_t[:, kt, :],
                            start=(kt == 0), stop=(kt == NKD - 1),
                        )
                for sub in range(2):
                    nc.vector.tensor_copy(
                        v_heads[:, st + sub, :, 0:HD],
                        ps[:, sub * CH:(sub + 1) * CH]
                        .rearrange("p (h c) -> p h c", c=HD)[:],
                    )
            return emit

        # never issue mid-run DMAs on nc.scalar: the ACT queue paces the
        # attention phases and a 667ns DGE config there stalls the exp chain
        out_eng = [nc.sync, nc.gpsimd]

        def out_unit(st):
            def emit():
                ps = psc.tile([P, 2 * CH], F32, tag="sc")
                for sub in range(2):
                    for j in range(NJ):
                        nc.tensor.matmul(
                            ps[:, sub * CH:(sub + 1) * CH],
                            ctxT_t[:, j, st * P:(st + 1) * P],
                            wd_t[:, j, sub * CH:(sub + 1) * CH],
                            start=(j == 0), stop=(j == NJ - 1),
                        )
                ost = ostg.tile([P, 2 * CH], BF16, tag="ost")
                nc.vector.tensor_copy(ost[:], ps[:])
                # split the store across two engine queues so back-to-back
                # units' stores pipeline instead of serializing on sync
                e0 = out_eng[st % 2]
                e1 = out_eng[(st + 1) % 2]
                e0.dma_start(outp[st * P:(st + 1) * P, 0:CH], ost[:, 0:CH])
                e1.dma_start(outp[st * P:(st + 1) * P, CH:2 * CH],
                             ost[:, CH:2 * CH])
            return emit

        queue = []

        def fill(n=1):
            for _ in range(n):
                if queue:
                    queue.pop(0)()

        drain_eng = [nc.sync, nc.gpsimd]

        # ---- one head pair (2j, 2j+1) over one q-half ----
        def attention_pair(j, half, fills=(), out_sts=None):
            lo, hi = HW * half, HW * (half + 1)
            hE, hO = 2 * j, 2 * j + 1
            ctxE = pcx.tile([HD + 1, HW], F32, tag="ctx")
            ctxO = pcx.tile([HD + 1, HW], F32, tag="ctx")
            nkt = (half + 1) * (NKT // 2)
            # global last key tile contributing to 512-chunk c of this half
            kt_last = [min(nkt - 1, (lo + (c + 1) * CH) // P - 1)
                       for c in range(NC_H)]

            def drain_chunk(c):
                # drain + normalize chunk c of both heads' ctx psum.
                # rowsum rides as psum row HD: invert it straight out of
                # psum into row 64 of the broadcast tile (f32, no staging
                # DMA), broadcast down, one multiply.
                g0, g1 = lo + c * CH, lo + (c + 1) * CH
                eng = drain_eng[(2 * j + half + c) % 2]
                for h, ctxp, pb in ((hE, ctxE, 0), (hO, ctxO, 64)):
                    # NB: partition_broadcast ucode reads the tile's physical
                    # partition 0 (not the AP base), and custom DVE ops can't
                    # read PSUM — so stage the rowsum to p0 with a plain
                    # cross-partition tensor_copy first.
                    bc = bcp.tile([HD, CH], F32, tag="bc")
                    nc.vector.tensor_copy(
                        bc[0:1, :], ctxp[HD:HD + 1, c * CH:(c + 1) * CH])
                    nc.vector.reciprocal_approx_fast(bc[0:1, :], bc[0:1, :])
                    nc.gpsimd.partition_broadcast(bc[:], bc[0:1, :])
                    c65 = cstg.tile([HD + 1, CH], BF16, tag="c64")
                    nc.vector.tensor_copy(
                        c65[0:HD, :], ctxp[0:HD, c * CH:(c + 1) * CH])
                    if pb == 0:
                        nc.vector.tensor_mul(
                            ctxT_t[0:HD, j, g0:g1], c65[0:HD, :], bc[0:HD, :])
                    else:
                        nc.vector.tensor_mul(
                            c65[0:HD, :], c65[0:HD, :], bc[0:HD, :])
                        eng.dma_start(
                            ctxT_t[pb:pb + HD, j, g0:g1], c65[0:HD, :])

            def emit_ctx(kt, q0, pe, po):
                for h, ctxp, pts in ((hE, ctxE, pe), (hO, ctxO, po)):
                    for c in range(NC_H):
                        c0g, c1g = lo + c * CH, lo + (c + 1) * CH
                        s = max(q0, c0g)
                        if s >= c1g:
                            continue
                        nc.tensor.matmul(
                            ctxp[:, s - lo:c1g - lo],
                            v_t[:, kt, h * (HD + 1):(h + 1) * (HD + 1)],
                            pts[:, s - q0:c1g - q0],
                            start=(kt == 0), stop=(kt == kt_last[c]),
                            skip_group_check=True,
                        )

            pend = None
            pend_drain = []
            for kt in range(nkt):
                q0 = max(P * kt, lo)
                width = hi - q0
                pe = ppool.tile([P, HW], BF16, tag="pts")
                po = ppool.tile([P, HW], BF16, tag="pts")
                for pb, pts in ((0, pe), (64, po)):
                    sps = psc.tile([P, 2 * CH], F32, tag="sc")
                    for c in range(0, width, CH):
                        cw = min(CH, width - c)
                        nc.tensor.matmul(
                            sps[:, c:c + cw],
                            kT_t[pb:pb + HD, j, P * kt:P * (kt + 1)],
                            qT_t[pb:pb + HD, j, q0 + c:q0 + c + cw],
                            start=True, stop=True,
                        )
                    nc.scalar.activation(
                        pts[:, 0:width], sps[:, 0:width], Exp,
                        scale=1.0 / np.sqrt(HD),
                    )
                    if q0 == P * kt:  # diagonal tile: causal mask
                        nc.vector.tensor_mul(pts[:, 0:P], pts[:, 0:P], mask_t[:])
                if kt in fills:
                    fill(1)
                # drains + dependent out-units from the previous kt: emitted
                # here (one kt late) so the PE queue isn't head-of-line
                # blocked waiting on the drain chain
                for c in pend_drain:
                    drain_chunk(c)
                    if out_sts is not None:
                        for st in out_sts[c]:
                            queue.append(out_unit(st))
                pend_drain = []
                if pend is not None:
                    emit_ctx(*pend)
                pend_drain = [c for c in range(NC_H)
                              if kt_last[c] == (kt - 1 if pend else -1)]
                pend = (kt, q0, pe, po)
            emit_ctx(*pend)
            for c in range(NC_H):
                if kt_last[c] >= nkt - 2:
                    drain_chunk(c)
                    if out_sts is not None:
                        for st in out_sts[c]:
                            queue.append(out_unit(st))

        # ---- PE warmup: junk matmuls on a memset tile (no DMA dependency,
        # starts right after the framework preamble) to ramp the PE pstate
        # and flip the HAM clock gate to 8/8 before the real work arrives ----
        js = cpool.tile([P, P], BF16, tag="js")
        nc.vector.memset(js[:], 0.01)
        junk = psc.tile([P, 2 * CH], F32, tag="sc")
        for i in range(48):
            nc.tensor.matmul(
                junk[:, (i % 4) * P:(i % 4 + 1) * P], js[:], js[:],
                start=True, stop=True, skip_group_check=True,
            )

        # ---- upfront: dim-block 0 n0 projections + first V tiles ----
        qk_unit(qT_t, wq_t, 0, 0)()
        qk_unit(kT_t, wk_t, 0, 0)()
        v_unit(0)()
        v_unit(2)()

        # filler queue, in need order
        queue.append(qk_unit(qT_t, wq_t, 0, HW))
        queue.append(qk_unit(kT_t, wk_t, 0, HW))
        queue.append(v_unit(4))
        queue.append(v_unit(6))
        for st in range(NKT // 2, NKT, 2):
            queue.append(v_unit(st))
        for j in range(1, NJ):
            queue.append(qk_unit(qT_t, wq_t, j, 0))
            queue.append(qk_unit(kT_t, wk_t, j, 0))
            queue.append(qk_unit(qT_t, wq_t, j, HW))
            queue.append(qk_unit(kT_t, wk_t, j, HW))

        # phase order: p3h0 early so half-0 output-proj work unblocks
        # in time to fill the late ACT-bound phases (HAM warmth)
        plan = [
            (0, 0, (1, 3, 5, 7), None),
            (0, 1, (1, 3, 5, 8, 11, 14), None),
            (1, 0, (1, 5), None),
            (1, 1, (1, 5, 8, 11), None),
            (2, 0, (1, 5), None),
            (3, 0, (1, 5), None),
            (2, 1, (1, 3, 5, 8, 11, 14), None),
            # final phase: queue each half-1 out-unit group as soon as its
            # ctx chunk (last needed input) is normalized
            (3, 1, (1, 3, 5, 8, 10, 12, 13, 14, 15),
             {0: range(8, 12), 1: range(12, 16)}),
        ]
        for j, half, fills, out_sts in plan:
            attention_pair(j, half, fills, out_sts)
            if (j, half) == (3, 0):
                # all half-0 ctx normalized: queue its output proj
                for st in range(NKT // 2):
                    queue.append(out_unit(st))
        fill(len(queue))

        if dbg is not None:
            nc.sync.dma_start(dbg["dqT"], qT_t[:])
            nc.sync.dma_start(dbg["dkT"], kT_t[:])
            nc.sync.dma_start(dbg["dv"], v_t[:])
            nc.sync.dma_start(dbg["dctxT"], ctxT_t[:])


def _compile():
    global _COMPILED
    if _COMPILED is not None:
        return _COMPILED
    nc = bacc.Bacc("TRN2", target_bir_lowering=False, debug=False,
                   num_devices=NCORES)
    aps = {
        "xT": nc.dram_tensor("xT", [D, S], BF16, kind="ExternalInput").ap(),
        "wq": nc.dram_tensor("wq", [D, DG], BF16, kind="ExternalInput").ap(),
        "wk": nc.dram_tensor("wk", [D, DG], BF16, kind="ExternalInput").ap(),
        "wv": nc.dram_tensor("wv", [D, DG], BF16, kind="ExternalInput").ap(),
        "wd": nc.dram_tensor("wd", [DG, D], BF16, kind="ExternalInput").ap(),
        "mask": nc.dram_tensor("mask", [P, P], BF16, kind="ExternalInput").ap(),
        "outp": nc.dram_tensor("outp", [S, D], BF16, kind="ExternalOutput").ap(),
    }
    with tile.TileContext(nc) as tc:
        _build_body(tc, aps)
    nc.compile()
    _COMPILED = nc
    return nc


def _host_shards(x, w_qkv):
    """Per-core input dicts (bf16)."""
    xb = [np.ascontiguousarray(x[b].T).astype(NPBF16) for b in range(B)]
    mask = np.triu(np.ones((P, P), dtype=np.float32)).astype(NPBF16)
    w = w_qkv.reshape(D, H, 3, HD)  # col = h*192 + t*64 + d
    shards = []
    for c in range(NCORES):
        b, g = c // 2, c % 2
        hs = slice(g * GH, (g + 1) * GH)
        shards.append({
            "xT": xb[b],
            "wq": np.ascontiguousarray(
                w[:, hs, 0, :].reshape(D, DG)).astype(NPBF16),
            "wk": np.ascontiguousarray(
                w[:, hs, 1, :].reshape(D, DG)).astype(NPBF16),
            "wv": np.ascontiguousarray(
                w[:, hs, 2, :].reshape(D, DG)).astype(NPBF16),
            "wd": None,  # filled by caller (needs w_dense)
            "mask": mask,
        })
    return shards


def _reference_fallback(x, w_qkv, b_qkv, w_dense, b_dense):
    qkv = x @ w_qkv + b_qkv
    b, s, d = x.shape
    qkv = qkv.reshape(b, s, H, 3 * HD).transpose(0, 2, 1, 3)
    q, k, v = np.split(qkv, 3, axis=-1)
    scores = np.einsum("bhqd,bhkd->bhqk", q, k) / np.sqrt(HD)
    causal = np.tril(np.ones((s, s), dtype=bool))[None, None]
    scores = np.where(causal, scores, -10000.0)
    scores -= scores.max(axis=-1, keepdims=True)
    p = np.exp(scores)
    p /= p.sum(axis=-1, keepdims=True)
    ctx = np.einsum("bhqk,bhkd->bhqd", p, v)
    ctx = ctx.transpose(0, 2, 1, 3).reshape(b, s, d)
    return (ctx @ w_dense + b_dense).astype(np.float32)


def kernel(x, w_qkv, b_qkv, w_dense, b_dense, _want_trace=False):
    x = np.asarray(x, dtype=np.float32)
    w_qkv = np.asarray(w_qkv, dtype=np.float32)
    b_qkv = np.asarray(b_qkv, dtype=np.float32)
    w_dense = np.asarray(w_dense, dtype=np.float32)
    b_dense = np.asarray(b_dense, dtype=np.float32)

    if np.abs(b_qkv).max() > 0:
        # qkv bias is zero in the problem spec; general path for safety
        return _reference_fallback(x, w_qkv, b_qkv, w_dense, b_dense)

    nc = _compile()
    shards = _host_shards(x, w_qkv)
    for c in range(NCORES):
        g = c % 2
        shards[c]["wd"] = np.ascontiguousarray(
            w_dense[g * DG:(g + 1) * DG, :]).astype(NPBF16)

    res = bass_utils.run_bass_kernel_spmd(
        nc, shards, core_ids=list(range(NCORES)), trace=_want_trace,
    )
    out = np.empty((B, S, D), dtype=np.float32)
    for b in range(B):
        out[b] = (res.results[2 * b]["outp"].astype(np.float32)
                  + res.results[2 * b + 1]["outp"].astype(np.float32))
    out += b_dense[None, None, :]
    if _want_trace:
        return out, res
    return out
